# revision 18
# baseline (speedup 1.0000x reference)
"""Trainium2 Bass kernel for nn_Disentangle_causal.

Math (per batch b):
  s  = ques + conc                               [L, H]
  qP = s[1:] @ Wq.T + bq  (for qcc / qct pairs)  [S, H]
  qc = Q @ cq_w.T + cq_b, kc = X @ ck_w.T + ck_b, etc.
  A_c  = (qc kc^T) * SCALE ; P_cc = (qcc_q qcc_k^T) * SCALE
  ac   = A_c * P_cc        -> causal_score  = softmax(mask(ac))
  A_t, P_ct likewise; at = 1 - sigmoid(A_t * P_ct) -> trivial_score
  g ~ gumbel(key 42, [B,S,2,S]);  idx = argmax_i(score_i + g_i)
  causal_mask = (idx==0) & ~mask ; trivial_mask = (idx==1) & ~mask

Device reformulation:
  1 - sigmoid(x) = 0.5 - 0.5*tanh(x/2)  (tanh shares ACT table set with exp)
  causal = ((e0/s0 - e1/s1) >= d) with d = g1-g0 (masked entries -> +BIG)
  trivial = M01 - causal  (M01 = 0/1 lower-tri mask)
SCALE is folded into the q-side weights/biases on the host. Only the
lower-triangular block-columns are computed; outputs are pre-zeroed by
the runtime, so masked blocks are never touched.
"""

import sys
import types

import numpy as np

if "/opt/trn_rl_repo" not in sys.path:
    sys.path.insert(0, "/opt/trn_rl_repo")

B, L, H = 64, 512, 256
S = L - 1
NCORE = 8
BPC = B // NCORE  # batches per core
SCALE = float(H) ** -0.5
BIG = np.float32(1.0e30)

# q-row tiles: (row_start, row_end, active_width)
QTILES = [(0, 128, 128), (128, 256, 256), (256, 384, 384), (384, 511, 511)]

_CACHE: dict = {}


def _install_ntff_hook():
    """Make trace=True work under axon (antenv.axon_hooks is not shipped)."""
    try:
        import antenv

        if "antenv.axon_hooks" in sys.modules:
            return
        hooks = types.ModuleType("antenv.axon_hooks")
        _hook = [None]
        hooks.set_axon_ntff_profile_hook = lambda h: _hook.__setitem__(0, h)
        hooks.get_axon_ntff_profile_hook = lambda: _hook[0]
        sys.modules["antenv.axon_hooks"] = hooks
        antenv.axon_hooks = hooks
        from trn_agent_boot.trn_boot import _ntff_profile_via_ctypes

        hooks.set_axon_ntff_profile_hook(
            _ntff_profile_via_ctypes("/opt/axon/libaxon_pjrt.so")
        )
    except Exception:
        pass


def _build_bass(use_f32r: bool):
    import concourse.mybir as mybir
    import concourse.tile as tile
    from concourse.bacc import Bacc
    from concourse.dve_ops import TENSOR_MASK_REDUCE as TMR

    dt = mybir.dt
    f32 = dt.float32
    AF = mybir.ActivationFunctionType
    OP = mybir.AluOpType

    nc = Bacc("TRN2", target_bir_lowering=False)

    mmdt = dt.float32r if use_f32r else f32
    qT = nc.dram_tensor("qT", [BPC, H, L], mmdt, kind="ExternalInput")
    xT = nc.dram_tensor("xT", [BPC, H, L], mmdt, kind="ExternalInput")
    sT = nc.dram_tensor("sT", [BPC, H, L], mmdt, kind="ExternalInput")
    dg = nc.dram_tensor("dg", [BPC, S, S], f32, kind="ExternalInput")
    wts = nc.dram_tensor("wts", [8, H, H], mmdt, kind="ExternalInput")
    bss = nc.dram_tensor("bss", [8, H, 1], f32, kind="ExternalInput")
    m01d = nc.dram_tensor("m01d", [4, 128, S], f32, kind="ExternalInput")
    rld = nc.dram_tensor("rld", [4, 128, 1], f32, kind="ExternalInput")
    out_c = nc.dram_tensor("out_c", [BPC, S, S], f32, kind="ExternalOutput")
    out_t = nc.dram_tensor("out_t", [BPC, S, S], f32, kind="ExternalOutput")

    def mm(ap):
        return ap

    with tile.TileContext(nc) as tc:
        with (
            tc.tile_pool(name="consts", bufs=1) as cpool,
            tc.tile_pool(name="inp", bufs=2) as ipool,
            tc.tile_pool(name="proj", bufs=2) as ppool,
            tc.tile_pool(name="mid", bufs=2) as mpool,
            tc.tile_pool(name="small", bufs=3) as spool,
            tc.tile_pool(name="psA", bufs=1, space="PSUM") as psA,
            tc.tile_pool(name="psB", bufs=3, space="PSUM") as psB,
        ):
            wtiles = []
            btiles = []
            for j in range(8):
                w0 = cpool.tile([128, H], mmdt, tag=f"w{j}0")
                nc.sync.dma_start(w0[:], wts[j, 0:128, :])
                w1 = cpool.tile([128, H], mmdt, tag=f"w{j}1")
                nc.sync.dma_start(w1[:], wts[j, 128:256, :])
                b0 = cpool.tile([128, 1], f32, tag=f"b{j}0")
                nc.sync.dma_start(b0[:], bss[j, 0:128, :])
                b1 = cpool.tile([128, 1], f32, tag=f"b{j}1")
                nc.sync.dma_start(b1[:], bss[j, 128:256, :])
                wtiles.append((w0, w1))
                btiles.append((b0, b1))
            m01 = []
            rl = []
            for i in range(4):
                m = cpool.tile([128, S], f32, tag=f"m01_{i}")
                nc.sync.dma_start(m[:], m01d[i])
                r = cpool.tile([128, 1], f32, tag=f"rl_{i}")
                nc.sync.dma_start(r[:], rld[i])
                m01.append(m)
                rl.append(r)
            half = cpool.tile([128, 1], f32, tag="half")
            nc.vector.memset(half[:], 0.5)

            for b in range(BPC):
                qt, xt = [], []
                for k in range(2):
                    t = ipool.tile([128, L], mmdt, tag=f"qt{k}")
                    nc.sync.dma_start(t[:], qT[b, 128 * k : 128 * (k + 1), :])
                    qt.append(t)
                    t = ipool.tile([128, L], mmdt, tag=f"xt{k}")
                    nc.sync.dma_start(t[:], xT[b, 128 * k : 128 * (k + 1), :])
                    xt.append(t)
                st = []
                for k in range(2):
                    t = ipool.tile([128, L], mmdt, tag=f"st{k}")
                    nc.sync.dma_start(t[:], sT[b, 128 * k : 128 * (k + 1), :])
                    st.append(t)

                # projection order: cq ck tq tk qcc_q qcc_k qct_q qct_k
                # All projections run over the full (padded) 512 columns; the
                # s[:,1:] / s[:,:-1] shifts are applied when slicing the
                # projection outputs for the score matmuls.
                srcs = [qt, xt, qt, xt, st, st, st, st]
                projs = []
                for j in range(8):
                    pr = []
                    for mh in range(2):
                        ps = psB.tile([128, L], f32, tag="psproj")
                        nc.tensor.matmul(
                            ps[:, :],
                            mm(wtiles[j][0][:, 128 * mh : 128 * (mh + 1)]),
                            mm(srcs[j][0][:, :]),
                            start=True,
                            stop=False,
                        )
                        nc.tensor.matmul(
                            ps[:, :],
                            mm(wtiles[j][1][:, 128 * mh : 128 * (mh + 1)]),
                            mm(srcs[j][1][:, :]),
                            start=False,
                            stop=True,
                        )
                        sb = ppool.tile([128, L + 2], mmdt, tag=f"proj{j}{mh}")
                        nc.scalar.activation(
                            sb[:, 0:L], ps[:], AF.Identity,
                            bias=btiles[j][mh][:], scale=1.0,
                        )
                        nc.gpsimd.memset(sb[:, L : L + 2].bitcast(dt.uint32), 0)
                        pr.append(sb)
                    projs.append(pr)

                p_cq, p_ck, p_tq, p_tk, p_ccq, p_cck, p_ctq, p_ctk = projs

                for i, (rs, re, W) in enumerate(QTILES):
                    M = re - rs
                    Me = 128          # padded stationary width (rows computed)
                    We = (W + 1) & ~1  # even moving width

                    def score(ql, qoff, kl, tag):
                        ps = psA.tile([128, L], f32, tag=tag)
                        nc.tensor.matmul(
                            ps[:Me, :We],
                            mm(ql[0][:, rs + qoff : rs + qoff + Me]),
                            mm(kl[0][:, 0:We]),
                            start=True,
                            stop=False,
                        )
                        nc.tensor.matmul(
                            ps[:Me, :We],
                            mm(ql[1][:, rs + qoff : rs + qoff + Me]),
                            mm(kl[1][:, 0:We]),
                            start=False,
                            stop=True,
                        )
                        return ps

                    # Q/X-side projections index q directly; s-side q
                    # projections are shifted by one (q = s[1:] slice).
                    A_c = score(p_cq, 0, p_ck, "scA")
                    P_cc = score(p_ccq, 1, p_cck, "scB")
                    A_t = score(p_tq, 0, p_tk, "scC")
                    P_ct = score(p_ctq, 1, p_ctk, "scD")

                    pcc = mpool.tile([128, S], f32, tag="pcc")
                    nc.scalar.activation(pcc[:M, :W], P_cc[:M, :W], AF.Copy)
                    pct = mpool.tile([128, S], f32, tag="pct")
                    nc.scalar.activation(pct[:M, :W], P_ct[:M, :W], AF.Copy)

                    p0 = mpool.tile([128, S], f32, tag="p0")
                    nc.vector.tensor_tensor(
                        p0[:M, :W], A_c[:M, :W], pcc[:M, :W], OP.mult
                    )
                    p1 = mpool.tile([128, S], f32, tag="p1")
                    nc.vector.tensor_tensor(
                        p1[:M, :W], A_t[:M, :W], pct[:M, :W], OP.mult
                    )

                    # causal branch: mask -> -FLT_MAX, rowmax, exp+sum
                    p0m = mpool.tile([128, S], f32, tag="p0m")
                    mx0 = spool.tile([128, 1], f32, tag="mx0")
                    nc.vector._custom_dve(
                        TMR,
                        out=p0m[:M, :W],
                        in0=p0[:M, :W],
                        in1=rl[i][:M, :],
                        s0=0.0,
                        s1=-3.4e38,
                        imm2=1.0,
                        accum_out=mx0[:M, :],
                    )
                    nmax0 = spool.tile([128, 1], f32, tag="nmax")
                    nc.vector.tensor_scalar_mul(nmax0[:M, :], mx0[:M, :], -1.0)
                    sums = spool.tile([128, 2], f32, tag="sums")
                    e0 = mpool.tile([128, S], f32, tag="e0")
                    nc.scalar.activation(
                        e0[:M, :W],
                        p0m[:M, :W],
                        AF.Exp,
                        bias=nmax0[:M, :],
                        scale=1.0,
                        accum_out=sums[:M, 0:1],
                    )

                    # trivial branch: 1-sigmoid(x) = 0.5 + 0.5*tanh(-x/2)
                    t1 = mpool.tile([128, S], f32, tag="t1")
                    nc.scalar.activation(t1[:M, :W], p1[:M, :W], AF.Tanh, scale=-0.5)
                    p1m = mpool.tile([128, S], f32, tag="p1m")
                    nc.vector._custom_dve(
                        TMR,
                        out=p1m[:M, :W],
                        in0=t1[:M, :W],
                        in1=rl[i][:M, :],
                        s0=0.0,
                        s1=-3.4e38,
                        imm2=0.5,
                    )
                    e1 = mpool.tile([128, S], f32, tag="e1")
                    nc.scalar.activation(
                        e1[:M, :W],
                        p1m[:M, :W],
                        AF.Exp,
                        bias=half[:M, :],
                        scale=1.0,
                        accum_out=sums[:M, 1:2],
                    )

                    rec = spool.tile([128, 2], f32, tag="rec")
                    nc.vector.reciprocal(rec[:M, :], sums[:M, :])

                    c1 = mpool.tile([128, S], f32, tag="c1")
                    nc.vector.tensor_scalar_mul(c1[:M, :W], e1[:M, :W], rec[:M, 1:2])

                    dgt = mpool.tile([128, S], f32, tag="dgt")
                    nc.sync.dma_start(dgt[:M, :W], dg[b, rs:re, 0:W])

                    diff = mpool.tile([128, S], f32, tag="diff")
                    nc.vector.scalar_tensor_tensor(
                        diff[:M, :W],
                        e0[:M, :W],
                        rec[:M, 0:1],
                        c1[:M, :W],
                        OP.mult,
                        OP.subtract,
                    )
                    cz = mpool.tile([128, S], f32, tag="cz")
                    nc.vector.tensor_tensor(
                        cz[:M, :W], diff[:M, :W], dgt[:M, :W], OP.is_ge
                    )
                    tz = mpool.tile([128, S], f32, tag="tz")
                    nc.vector.tensor_tensor(
                        tz[:M, :W], m01[i][:M, :W], cz[:M, :W], OP.subtract
                    )

                    nc.sync.dma_start(out_c[b, rs:re, 0:W], cz[:M, :W])
                    nc.sync.dma_start(out_t[b, rs:re, 0:W], tz[:M, :W])

    nc.compile()
    return nc


def _get_nc(use_f32r: bool):
    key = ("nc", use_f32r)
    if key not in _CACHE:
        _CACHE[key] = _build_bass(use_f32r)
    return _CACHE[key]


def _gumbel_diff():
    """d[b,q,k] = g1 - g0 for the fixed key-42 gumbel draw of the reference."""
    if "gd" not in _CACHE:
        import jax

        g = jax.random.gumbel(jax.random.key(42), (B, S, 2, S), dtype=np.float32)
        _CACHE["gd"] = np.asarray(g[:, :, 1, :] - g[:, :, 0, :], dtype=np.float32)
    return _CACHE["gd"]


def _host_fallback(kw):
    """Exact reference recomputation on host (only used if the masks are not
    the expected causal/no-padding pattern)."""
    import jax
    import jax.numpy as jnp

    def qc_score(ques, conc, qw, qb, kw_, kb):
        s = ques + conc
        q = s[:, 1:] @ qw.T + qb
        k = s[:, :-1] @ kw_.T + kb
        return jnp.einsum("bqh,bkh->bqk", q, k) * SCALE

    def masked(a, att, kpm):
        neg = jnp.float32(-jnp.inf)
        a = jnp.where(att, neg, a)
        return jnp.where(kpm[:, None, :], neg, a)

    att, kpm = kw["att_mask"], kw["key_padding_mask"]
    qc_c = qc_score(kw["ques_state"], kw["conc_state"], kw["qcc_q_w"], kw["qcc_q_b"],
                    kw["qcc_k_w"], kw["qcc_k_b"])
    qc_t = qc_score(kw["ques_state"], kw["conc_state"], kw["qct_q_w"], kw["qct_q_b"],
                    kw["qct_k_w"], kw["qct_k_b"])
    qc = kw["Q_state"] @ kw["cq_w"].T + kw["cq_b"]
    kc = kw["X_state"] @ kw["ck_w"].T + kw["ck_b"]
    ac = jnp.einsum("bqh,bkh->bqk", qc, kc) * SCALE * qc_c
    c_score = jax.nn.softmax(masked(ac, att, kpm), axis=-1)
    qt = kw["Q_state"] @ kw["tq_w"].T + kw["tq_b"]
    kt = kw["X_state"] @ kw["tk_w"].T + kw["tk_b"]
    at = jnp.einsum("bqh,bkh->bqk", qt, kt) * SCALE * qc_t
    at = 1.0 - jax.nn.sigmoid(at)
    t_score = jax.nn.softmax(masked(at, att, kpm), axis=-1)
    score = jnp.stack([c_score, t_score], axis=2)
    g = jax.random.gumbel(jax.random.key(42), score.shape, dtype=score.dtype)
    y = jax.nn.softmax(score + g, axis=2)
    idx = jnp.argmax(y, axis=2)
    causal = jnp.where(att, 0.0, (idx == 0).astype(np.float32))
    trivial = jnp.where(att, 0.0, (idx == 1).astype(np.float32))
    return np.asarray(causal), np.asarray(trivial)


def kernel(**inputs):
    import os

    Q = np.asarray(inputs["Q_state"], dtype=np.float32)
    X = np.asarray(inputs["X_state"], dtype=np.float32)
    ques = np.asarray(inputs["ques_state"], dtype=np.float32)
    conc = np.asarray(inputs["conc_state"], dtype=np.float32)
    att = np.asarray(inputs["att_mask"])
    kpm = np.asarray(inputs["key_padding_mask"])

    triu = np.triu(np.ones((S, S), dtype=bool), k=1)
    if kpm.any() or not (att == triu[None]).all():
        return _host_fallback(inputs)

    use_f32r = os.environ.get("KBENCH_F32R", "0") == "1"
    trace = os.environ.get("KBENCH_TRACE", "0") == "1"
    _install_ntff_hook()

    names = ["cq", "ck", "tq", "tk", "qcc_q", "qcc_k", "qct_q", "qct_k"]
    q_side = {"cq", "tq", "qcc_q", "qct_q"}
    wts = np.empty((8, H, H), np.float32)
    bss = np.empty((8, H, 1), np.float32)
    for j, nm in enumerate(names):
        w = np.asarray(inputs[nm + "_w"], dtype=np.float32)
        bvec = np.asarray(inputs[nm + "_b"], dtype=np.float32)
        if nm in q_side:
            w = w * SCALE
            bvec = bvec * SCALE
        wts[j] = np.ascontiguousarray(w.T)  # [h_in, h_out]
        bss[j, :, 0] = bvec

    # masks / rowlen consts
    m01 = np.zeros((4, 128, S), np.float32)
    rld = np.zeros((4, 128, 1), np.float32)
    for i, (rs, re, W) in enumerate(QTILES):
        for p in range(re - rs):
            q = rs + p
            m01[i, p, : q + 1] = 1.0
            rld[i, p, 0] = q + 1

    d = _gumbel_diff()
    s_full = ques + conc

    qT = np.zeros((B, H, L), np.float32)
    qT[:, :, :S] = Q.transpose(0, 2, 1)
    xT = np.zeros((B, H, L), np.float32)
    xT[:, :, :S] = X.transpose(0, 2, 1)
    sT = np.ascontiguousarray(s_full.transpose(0, 2, 1))
    dgm = np.where(triu[None], BIG, d).astype(np.float32)

    nc = _get_nc(use_f32r)
    from concourse.bass_utils import run_bass_kernel_spmd

    in_maps = []
    for c in range(NCORE):
        sl = slice(c * BPC, (c + 1) * BPC)
        in_maps.append(
            {
                "qT": qT[sl],
                "xT": xT[sl],
                "sT": sT[sl],
                "dg": dgm[sl],
                "wts": wts,
                "bss": bss,
                "m01d": m01,
                "rld": rld,
            }
        )

    kwargs = {}
    if trace:
        n = _CACHE.get("trace_n", 0)
        _CACHE["trace_n"] = n + 1
        tdir = os.environ.get("KBENCH_TRACE_DIR", "/tmp/kbench_trace") + f"_{n}"
        os.makedirs(tdir, exist_ok=True)
        kwargs = {"trace": True, "tmpdir": tdir}
        _CACHE["trace_dir"] = tdir
    res = run_bass_kernel_spmd(nc, in_maps, core_ids=list(range(NCORE)), **kwargs)
    if trace:
        _CACHE["exec_time_ns"] = res.exec_time_ns
        _CACHE["mean_exec_time_ns"] = res.mean_exec_time_ns

    causal = np.empty((B, S, S), np.float32)
    trivial = np.empty((B, S, S), np.float32)
    for c in range(NCORE):
        sl = slice(c * BPC, (c + 1) * BPC)
        causal[sl] = res.results[c]["out_c"]
        trivial[sl] = res.results[c]["out_t"]
    return causal, trivial


# revision 22
# speedup vs baseline: 1.1356x; 1.1356x over previous
"""Trainium2 Bass kernel for nn_Disentangle_causal.

Math (per batch b):
  s  = ques + conc                               [L, H]
  qP = s[1:] @ Wq.T + bq  (for qcc / qct pairs)  [S, H]
  qc = Q @ cq_w.T + cq_b, kc = X @ ck_w.T + ck_b, etc.
  A_c  = (qc kc^T) * SCALE ; P_cc = (qcc_q qcc_k^T) * SCALE
  ac   = A_c * P_cc        -> causal_score  = softmax(mask(ac))
  A_t, P_ct likewise; at = 1 - sigmoid(A_t * P_ct) -> trivial_score
  g ~ gumbel(key 42, [B,S,2,S]);  idx = argmax_i(score_i + g_i)
  causal_mask = (idx==0) & ~mask ; trivial_mask = (idx==1) & ~mask

Device reformulation:
  1 - sigmoid(x) = 0.5 - 0.5*tanh(x/2)  (tanh shares ACT table set with exp)
  causal = ((e0/s0 - e1/s1) >= d) with d = g1-g0 (masked entries -> +BIG)
  trivial = M01 - causal  (M01 = 0/1 lower-tri mask)
SCALE is folded into the q-side weights/biases on the host. Only the
lower-triangular block-columns are computed; outputs are pre-zeroed by
the runtime, so masked blocks are never touched.
"""

import sys
import types

import numpy as np

if "/opt/trn_rl_repo" not in sys.path:
    sys.path.insert(0, "/opt/trn_rl_repo")

B, L, H = 64, 512, 256
S = L - 1
NCORE = 8
BPC = B // NCORE  # batches per core
SCALE = float(H) ** -0.5
BIG = np.float32(1.0e30)

# q-row tiles: (row_start, row_end, active_width)
QTILES = [(0, 128, 128), (128, 256, 256), (256, 384, 384), (384, 511, 511)]

_CACHE: dict = {}


def _install_ntff_hook():
    """Make trace=True work under axon (antenv.axon_hooks is not shipped)."""
    try:
        import antenv

        if "antenv.axon_hooks" in sys.modules:
            return
        hooks = types.ModuleType("antenv.axon_hooks")
        _hook = [None]
        hooks.set_axon_ntff_profile_hook = lambda h: _hook.__setitem__(0, h)
        hooks.get_axon_ntff_profile_hook = lambda: _hook[0]
        sys.modules["antenv.axon_hooks"] = hooks
        antenv.axon_hooks = hooks
        from trn_agent_boot.trn_boot import _ntff_profile_via_ctypes

        hooks.set_axon_ntff_profile_hook(
            _ntff_profile_via_ctypes("/opt/axon/libaxon_pjrt.so")
        )
    except Exception:
        pass


def _build_bass(use_f32r: bool):
    import concourse.mybir as mybir
    import concourse.tile as tile
    from concourse.bacc import Bacc
    from concourse.dve_ops import TENSOR_MASK_REDUCE as TMR

    dt = mybir.dt
    f32 = dt.float32
    AF = mybir.ActivationFunctionType
    OP = mybir.AluOpType

    nc = Bacc("TRN2", target_bir_lowering=False)

    mmdt = dt.float32r if use_f32r else f32
    qT = nc.dram_tensor("qT", [BPC, H, L], mmdt, kind="ExternalInput")
    xT = nc.dram_tensor("xT", [BPC, H, L], mmdt, kind="ExternalInput")
    sT = nc.dram_tensor("sT", [BPC, H, L], mmdt, kind="ExternalInput")
    dg = nc.dram_tensor("dg", [BPC, L, L], f32, kind="ExternalInput")
    wts = nc.dram_tensor("wts", [8, H, H], mmdt, kind="ExternalInput")
    bss = nc.dram_tensor("bss", [128, 16], f32, kind="ExternalInput")
    m01d = nc.dram_tensor("m01d", [4, 128, L], f32, kind="ExternalInput")
    rld = nc.dram_tensor("rld", [128, 4], f32, kind="ExternalInput")
    out_c = nc.dram_tensor("out_c", [BPC, S, L], f32, kind="ExternalOutput")
    out_t = nc.dram_tensor("out_t", [BPC, S, L], f32, kind="ExternalOutput")

    def mm(ap):
        return ap

    with tile.TileContext(nc) as tc:
        with (
            tc.tile_pool(name="consts", bufs=1) as cpool,
            tc.tile_pool(name="inp", bufs=2) as ipool,
            tc.tile_pool(name="proj", bufs=2) as ppool,
            tc.tile_pool(name="mid", bufs=2) as mpool,
            tc.tile_pool(name="small", bufs=3) as spool,
            tc.tile_pool(name="psA", bufs=1, space="PSUM") as psA,
            tc.tile_pool(name="psB", bufs=3, space="PSUM") as psB,
        ):
            wtiles = []
            for j in range(8):
                w0 = cpool.tile([128, H], mmdt, tag=f"w{j}0")
                nc.sync.dma_start(w0[:], wts[j, 0:128, :])
                w1 = cpool.tile([128, H], mmdt, tag=f"w{j}1")
                nc.sync.dma_start(w1[:], wts[j, 128:256, :])
                wtiles.append((w0, w1))
            bpk = cpool.tile([128, 16], f32, tag="bpk")
            nc.sync.dma_start(bpk[:], bss[:, :])
            btiles = [(bpk[:, 2 * j : 2 * j + 1], bpk[:, 2 * j + 1 : 2 * j + 2])
                      for j in range(8)]
            m01 = []
            for i in range(4):
                m = cpool.tile([128, L], f32, tag=f"m01_{i}")
                nc.sync.dma_start(m[:], m01d[i])
                m01.append(m)
            rlt = cpool.tile([128, 4], f32, tag="rlt")
            nc.sync.dma_start(rlt[:], rld[:, :])
            rl = [rlt[:, i : i + 1] for i in range(4)]
            half = cpool.tile([128, 1], f32, tag="half")
            nc.vector.memset(half[:], 0.5)

            for b in range(BPC):
                qt, xt = [], []
                for k in range(2):
                    t = ipool.tile([128, L], mmdt, tag=f"qt{k}")
                    nc.sync.dma_start(t[:], qT[b, 128 * k : 128 * (k + 1), :])
                    qt.append(t)
                    t = ipool.tile([128, L], mmdt, tag=f"xt{k}")
                    nc.sync.dma_start(t[:], xT[b, 128 * k : 128 * (k + 1), :])
                    xt.append(t)
                st = []
                for k in range(2):
                    t = ipool.tile([128, L], mmdt, tag=f"st{k}")
                    nc.sync.dma_start(t[:], sT[b, 128 * k : 128 * (k + 1), :])
                    st.append(t)

                # projection order: cq ck tq tk qcc_q qcc_k qct_q qct_k
                # All projections run over the full (padded) 512 columns; the
                # s[:,1:] / s[:,:-1] shifts are applied when slicing the
                # projection outputs for the score matmuls.
                srcs = [qt, xt, qt, xt, st, st, st, st]
                projs = []
                for j in range(8):
                    pr = []
                    for mh in range(2):
                        ps = psB.tile([128, L], f32, tag="psproj")
                        nc.tensor.matmul(
                            ps[:, :],
                            mm(wtiles[j][0][:, 128 * mh : 128 * (mh + 1)]),
                            mm(srcs[j][0][:, :]),
                            start=True,
                            stop=False,
                        )
                        nc.tensor.matmul(
                            ps[:, :],
                            mm(wtiles[j][1][:, 128 * mh : 128 * (mh + 1)]),
                            mm(srcs[j][1][:, :]),
                            start=False,
                            stop=True,
                        )
                        sb = ppool.tile([128, L + 2], mmdt, tag=f"proj{j}{mh}")
                        if j in (1, 3, 5):
                            nc.vector.tensor_scalar_add(
                                sb[:, 0:L], ps[:], btiles[j][mh]
                            )
                        else:
                            nc.scalar.activation(
                                sb[:, 0:L], ps[:], AF.Identity,
                                bias=btiles[j][mh], scale=1.0,
                            )
                        nc.gpsimd.memset(sb[:, L : L + 2].bitcast(dt.uint32), 0)
                        pr.append(sb)
                    projs.append(pr)

                p_cq, p_ck, p_tq, p_tk, p_ccq, p_cck, p_ctq, p_ctk = projs

                for i, (rs, re, W) in enumerate(QTILES):
                    M = re - rs
                    Me = 128          # padded stationary width (rows computed)
                    We = (W + 1) & ~1  # even moving width

                    def score(ql, qoff, kl, tag):
                        ps = psA.tile([128, L], f32, tag=tag)
                        nc.tensor.matmul(
                            ps[:Me, :We],
                            mm(ql[0][:, rs + qoff : rs + qoff + Me]),
                            mm(kl[0][:, 0:We]),
                            start=True,
                            stop=False,
                        )
                        nc.tensor.matmul(
                            ps[:Me, :We],
                            mm(ql[1][:, rs + qoff : rs + qoff + Me]),
                            mm(kl[1][:, 0:We]),
                            start=False,
                            stop=True,
                        )
                        return ps

                    # Q/X-side projections index q directly; s-side q
                    # projections are shifted by one (q = s[1:] slice).
                    A_c = score(p_cq, 0, p_ck, "scA")
                    P_cc = score(p_ccq, 1, p_cck, "scB")
                    A_t = score(p_tq, 0, p_tk, "scC")
                    P_ct = score(p_ctq, 1, p_ctk, "scD")

                    pcc = mpool.tile([128, L], f32, tag="pcc")
                    nc.scalar.activation(pcc[:M, :We], P_cc[:M, :We], AF.Copy)
                    pct = mpool.tile([128, L], f32, tag="pct")
                    nc.vector.tensor_copy(pct[:M, :We], P_ct[:M, :We])

                    p0 = mpool.tile([128, L], f32, tag="p0")
                    nc.vector.tensor_tensor(
                        p0[:M, :We], A_c[:M, :We], pcc[:M, :We], OP.mult
                    )
                    p1 = mpool.tile([128, L], f32, tag="p1")
                    nc.vector.tensor_tensor(
                        p1[:M, :We], A_t[:M, :We], pct[:M, :We], OP.mult
                    )

                    # causal branch: mask -> -FLT_MAX, rowmax, exp+sum
                    p0m = mpool.tile([128, L], f32, tag="p0m")
                    mx0 = spool.tile([128, 1], f32, tag="mx0")
                    nc.vector._custom_dve(
                        TMR,
                        out=p0m[:M, :We],
                        in0=p0[:M, :We],
                        in1=rl[i][:M, :],
                        s0=0.0,
                        s1=-3.4e38,
                        imm2=1.0,
                        accum_out=mx0[:M, :],
                    )
                    nmax0 = spool.tile([128, 1], f32, tag="nmax")
                    nc.vector.tensor_scalar_mul(nmax0[:M, :], mx0[:M, :], -1.0)
                    sums = spool.tile([128, 2], f32, tag="sums")
                    e0 = mpool.tile([128, L], f32, tag="e0")
                    nc.scalar.activation(
                        e0[:M, :We],
                        p0m[:M, :We],
                        AF.Exp,
                        bias=nmax0[:M, :],
                        scale=1.0,
                        accum_out=sums[:M, 0:1],
                    )

                    # trivial branch: 1-sigmoid(x) = 0.5 + 0.5*tanh(-x/2)
                    t1 = mpool.tile([128, L], f32, tag="t1")
                    nc.scalar.activation(t1[:M, :We], p1[:M, :We], AF.Tanh, scale=-0.5)
                    p1m = mpool.tile([128, L], f32, tag="p1m")
                    nc.vector._custom_dve(
                        TMR,
                        out=p1m[:M, :We],
                        in0=t1[:M, :We],
                        in1=rl[i][:M, :],
                        s0=0.0,
                        s1=-3.4e38,
                        imm2=0.5,
                    )
                    e1 = mpool.tile([128, L], f32, tag="e1")
                    nc.scalar.activation(
                        e1[:M, :We],
                        p1m[:M, :We],
                        AF.Exp,
                        bias=half[:M, :],
                        scale=1.0,
                        accum_out=sums[:M, 1:2],
                    )

                    rec = spool.tile([128, 2], f32, tag="rec")
                    nc.vector.reciprocal(rec[:M, :], sums[:M, :])

                    c1 = mpool.tile([128, L], f32, tag="c1")
                    nc.vector.tensor_scalar_mul(c1[:M, :We], e1[:M, :We], rec[:M, 1:2])

                    dgt = mpool.tile([128, L], f32, tag="dgt")
                    nc.sync.dma_start(dgt[:M, :We], dg[b, rs:re, 0:We])

                    diff = mpool.tile([128, L], f32, tag="diff")
                    nc.vector.scalar_tensor_tensor(
                        diff[:M, :We],
                        e0[:M, :We],
                        rec[:M, 0:1],
                        c1[:M, :We],
                        OP.mult,
                        OP.subtract,
                    )
                    cz = mpool.tile([128, L], f32, tag="cz")
                    nc.vector.tensor_tensor(
                        cz[:M, :We], diff[:M, :We], dgt[:M, :We], OP.is_ge
                    )
                    tz = mpool.tile([128, L], f32, tag="tz")
                    nc.vector.tensor_tensor(
                        tz[:M, :We], m01[i][:M, :We], cz[:M, :We], OP.subtract
                    )

                    nc.sync.dma_start(out_c[b, rs:re, 0:We], cz[:M, :We])
                    nc.sync.dma_start(out_t[b, rs:re, 0:We], tz[:M, :We])

    nc.compile()
    return nc


def _get_nc(use_f32r: bool):
    key = ("nc", use_f32r)
    if key not in _CACHE:
        _CACHE[key] = _build_bass(use_f32r)
    return _CACHE[key]


def _gumbel_diff():
    """d[b,q,k] = g1 - g0 for the fixed key-42 gumbel draw of the reference."""
    if "gd" not in _CACHE:
        import jax

        g = jax.random.gumbel(jax.random.key(42), (B, S, 2, S), dtype=np.float32)
        _CACHE["gd"] = np.asarray(g[:, :, 1, :] - g[:, :, 0, :], dtype=np.float32)
    return _CACHE["gd"]


def _host_fallback(kw):
    """Exact reference recomputation on host (only used if the masks are not
    the expected causal/no-padding pattern)."""
    import jax
    import jax.numpy as jnp

    def qc_score(ques, conc, qw, qb, kw_, kb):
        s = ques + conc
        q = s[:, 1:] @ qw.T + qb
        k = s[:, :-1] @ kw_.T + kb
        return jnp.einsum("bqh,bkh->bqk", q, k) * SCALE

    def masked(a, att, kpm):
        neg = jnp.float32(-jnp.inf)
        a = jnp.where(att, neg, a)
        return jnp.where(kpm[:, None, :], neg, a)

    att, kpm = kw["att_mask"], kw["key_padding_mask"]
    qc_c = qc_score(kw["ques_state"], kw["conc_state"], kw["qcc_q_w"], kw["qcc_q_b"],
                    kw["qcc_k_w"], kw["qcc_k_b"])
    qc_t = qc_score(kw["ques_state"], kw["conc_state"], kw["qct_q_w"], kw["qct_q_b"],
                    kw["qct_k_w"], kw["qct_k_b"])
    qc = kw["Q_state"] @ kw["cq_w"].T + kw["cq_b"]
    kc = kw["X_state"] @ kw["ck_w"].T + kw["ck_b"]
    ac = jnp.einsum("bqh,bkh->bqk", qc, kc) * SCALE * qc_c
    c_score = jax.nn.softmax(masked(ac, att, kpm), axis=-1)
    qt = kw["Q_state"] @ kw["tq_w"].T + kw["tq_b"]
    kt = kw["X_state"] @ kw["tk_w"].T + kw["tk_b"]
    at = jnp.einsum("bqh,bkh->bqk", qt, kt) * SCALE * qc_t
    at = 1.0 - jax.nn.sigmoid(at)
    t_score = jax.nn.softmax(masked(at, att, kpm), axis=-1)
    score = jnp.stack([c_score, t_score], axis=2)
    g = jax.random.gumbel(jax.random.key(42), score.shape, dtype=score.dtype)
    y = jax.nn.softmax(score + g, axis=2)
    idx = jnp.argmax(y, axis=2)
    causal = jnp.where(att, 0.0, (idx == 0).astype(np.float32))
    trivial = jnp.where(att, 0.0, (idx == 1).astype(np.float32))
    return np.asarray(causal), np.asarray(trivial)


def kernel(**inputs):
    import os

    Q = np.asarray(inputs["Q_state"], dtype=np.float32)
    X = np.asarray(inputs["X_state"], dtype=np.float32)
    ques = np.asarray(inputs["ques_state"], dtype=np.float32)
    conc = np.asarray(inputs["conc_state"], dtype=np.float32)
    att = np.asarray(inputs["att_mask"])
    kpm = np.asarray(inputs["key_padding_mask"])

    triu = np.triu(np.ones((S, S), dtype=bool), k=1)
    if kpm.any() or not (att == triu[None]).all():
        return _host_fallback(inputs)

    use_f32r = os.environ.get("KBENCH_F32R", "0") == "1"
    trace = os.environ.get("KBENCH_TRACE", "0") == "1"
    _install_ntff_hook()

    names = ["cq", "ck", "tq", "tk", "qcc_q", "qcc_k", "qct_q", "qct_k"]
    q_side = {"cq", "tq", "qcc_q", "qct_q"}
    wts = np.empty((8, H, H), np.float32)
    bss = np.empty((128, 16), np.float32)  # [p, 2j+mh] = b_j[128*mh + p]
    for j, nm in enumerate(names):
        w = np.asarray(inputs[nm + "_w"], dtype=np.float32)
        bvec = np.asarray(inputs[nm + "_b"], dtype=np.float32)
        if nm in q_side:
            w = w * SCALE
            bvec = bvec * SCALE
        wts[j] = np.ascontiguousarray(w.T)  # [h_in, h_out]
        bss[:, 2 * j] = bvec[0:128]
        bss[:, 2 * j + 1] = bvec[128:256]

    # masks / rowlen consts (padded to L columns)
    m01 = np.zeros((4, 128, L), np.float32)
    rld = np.zeros((128, 4), np.float32)
    for i, (rs, re, W) in enumerate(QTILES):
        for p in range(re - rs):
            q = rs + p
            m01[i, p, : q + 1] = 1.0
            rld[p, i] = q + 1

    d = _gumbel_diff()
    s_full = ques + conc

    qT = np.zeros((B, H, L), np.float32)
    qT[:, :, :S] = Q.transpose(0, 2, 1)
    xT = np.zeros((B, H, L), np.float32)
    xT[:, :, :S] = X.transpose(0, 2, 1)
    sT = np.ascontiguousarray(s_full.transpose(0, 2, 1))
    dgm = np.full((B, L, L), BIG, np.float32)
    dgm[:, :S, :S] = np.where(triu[None], BIG, d)

    nc = _get_nc(use_f32r)
    from concourse.bass_utils import run_bass_kernel_spmd

    in_maps = []
    for c in range(NCORE):
        sl = slice(c * BPC, (c + 1) * BPC)
        in_maps.append(
            {
                "qT": qT[sl],
                "xT": xT[sl],
                "sT": sT[sl],
                "dg": dgm[sl],
                "wts": wts,
                "bss": bss,
                "m01d": m01,
                "rld": rld,
            }
        )

    kwargs = {}
    if trace:
        n = _CACHE.get("trace_n", 0)
        _CACHE["trace_n"] = n + 1
        tdir = os.environ.get("KBENCH_TRACE_DIR", "/tmp/kbench_trace") + f"_{n}"
        os.makedirs(tdir, exist_ok=True)
        kwargs = {"trace": True, "tmpdir": tdir}
        _CACHE["trace_dir"] = tdir
    res = run_bass_kernel_spmd(nc, in_maps, core_ids=list(range(NCORE)), **kwargs)
    if trace:
        _CACHE["exec_time_ns"] = res.exec_time_ns
        _CACHE["mean_exec_time_ns"] = res.mean_exec_time_ns

    causal = np.empty((B, S, S), np.float32)
    trivial = np.empty((B, S, S), np.float32)
    for c in range(NCORE):
        sl = slice(c * BPC, (c + 1) * BPC)
        causal[sl] = res.results[c]["out_c"][:, :, :S]
        trivial[sl] = res.results[c]["out_t"][:, :, :S]
    return causal, trivial


# revision 23
# speedup vs baseline: 1.7744x; 1.5626x over previous
"""Trainium2 Bass kernel for nn_Disentangle_causal.

Math (per batch b):
  s  = ques + conc                               [L, H]
  qP = s[1:] @ Wq.T + bq  (for qcc / qct pairs)  [S, H]
  qc = Q @ cq_w.T + cq_b, kc = X @ ck_w.T + ck_b, etc.
  A_c  = (qc kc^T) * SCALE ; P_cc = (qcc_q qcc_k^T) * SCALE
  ac   = A_c * P_cc        -> causal_score  = softmax(mask(ac))
  A_t, P_ct likewise; at = 1 - sigmoid(A_t * P_ct) -> trivial_score
  g ~ gumbel(key 42, [B,S,2,S]);  idx = argmax_i(score_i + g_i)
  causal_mask = (idx==0) & ~mask ; trivial_mask = (idx==1) & ~mask

Device reformulation:
  1 - sigmoid(x) = 0.5 - 0.5*tanh(x/2)  (tanh shares ACT table set with exp)
  causal = ((e0/s0 - e1/s1) >= d) with d = g1-g0 (masked entries -> +BIG)
  trivial = M01 - causal  (M01 = 0/1 lower-tri mask)
SCALE is folded into the q-side weights/biases on the host. Only the
lower-triangular block-columns are computed; outputs are pre-zeroed by
the runtime, so masked blocks are never touched.
"""

import sys
import types

import numpy as np

if "/opt/trn_rl_repo" not in sys.path:
    sys.path.insert(0, "/opt/trn_rl_repo")

B, L, H = 64, 512, 256
S = L - 1
NCORE = 8
BPC = B // NCORE  # batches per core
SCALE = float(H) ** -0.5
BIG = np.float32(1.0e30)

# q-row tiles: (row_start, row_end, active_width)
QTILES = [(0, 128, 128), (128, 256, 256), (256, 384, 384), (384, 511, 511)]

_CACHE: dict = {}


def _install_ntff_hook():
    """Make trace=True work under axon (antenv.axon_hooks is not shipped)."""
    try:
        import antenv

        if "antenv.axon_hooks" in sys.modules:
            return
        hooks = types.ModuleType("antenv.axon_hooks")
        _hook = [None]
        hooks.set_axon_ntff_profile_hook = lambda h: _hook.__setitem__(0, h)
        hooks.get_axon_ntff_profile_hook = lambda: _hook[0]
        sys.modules["antenv.axon_hooks"] = hooks
        antenv.axon_hooks = hooks
        from trn_agent_boot.trn_boot import _ntff_profile_via_ctypes

        hooks.set_axon_ntff_profile_hook(
            _ntff_profile_via_ctypes("/opt/axon/libaxon_pjrt.so")
        )
    except Exception:
        pass


def _build_bass(use_f32r: bool):
    import concourse.mybir as mybir
    import concourse.tile as tile
    from concourse.bacc import Bacc
    from concourse.dve_ops import TENSOR_MASK_REDUCE as TMR

    dt = mybir.dt
    f32 = dt.float32
    AF = mybir.ActivationFunctionType
    OP = mybir.AluOpType

    nc = Bacc("TRN2", target_bir_lowering=False)

    mmdt = dt.float32r if use_f32r else f32
    qT = nc.dram_tensor("qT", [BPC, H, L], mmdt, kind="ExternalInput")
    xT = nc.dram_tensor("xT", [BPC, H, L], mmdt, kind="ExternalInput")
    sT = nc.dram_tensor("sT", [BPC, H, L], mmdt, kind="ExternalInput")
    dg = nc.dram_tensor("dg", [BPC, L, L], f32, kind="ExternalInput")
    wts = nc.dram_tensor("wts", [8, H, H], mmdt, kind="ExternalInput")
    bss = nc.dram_tensor("bss", [128, 16], f32, kind="ExternalInput")
    m01d = nc.dram_tensor("m01d", [4, 128, L], f32, kind="ExternalInput")
    rld = nc.dram_tensor("rld", [128, 4], f32, kind="ExternalInput")
    out_c = nc.dram_tensor("out_c", [BPC, L, L], f32, kind="ExternalOutput")
    out_t = nc.dram_tensor("out_t", [BPC, L, L], f32, kind="ExternalOutput")

    def mm(ap):
        return ap

    with tile.TileContext(nc) as tc:
        with (
            tc.tile_pool(name="consts", bufs=1) as cpool,
            tc.tile_pool(name="inp", bufs=2) as ipool,
            tc.tile_pool(name="proj", bufs=2) as ppool,
            tc.tile_pool(name="mid", bufs=2) as mpool,
            tc.tile_pool(name="mid1", bufs=1) as m1pool,
            tc.tile_pool(name="small", bufs=3) as spool,
            tc.tile_pool(name="psA", bufs=1, space="PSUM") as psA,
            tc.tile_pool(name="psB", bufs=3, space="PSUM") as psB,
        ):
            wtiles = []
            for j in range(8):
                w0 = cpool.tile([128, H], mmdt, tag=f"w{j}0")
                nc.sync.dma_start(w0[:], wts[j, 0:128, :])
                w1 = cpool.tile([128, H], mmdt, tag=f"w{j}1")
                nc.sync.dma_start(w1[:], wts[j, 128:256, :])
                wtiles.append((w0, w1))
            bpk = cpool.tile([128, 16], f32, tag="bpk")
            nc.sync.dma_start(bpk[:], bss[:, :])
            btiles = [(bpk[:, 2 * j : 2 * j + 1], bpk[:, 2 * j + 1 : 2 * j + 2])
                      for j in range(8)]
            m01 = []
            for i in range(4):
                m = cpool.tile([128, L], f32, tag=f"m01_{i}")
                nc.sync.dma_start(m[:], m01d[i])
                m01.append(m)
            rlt = cpool.tile([128, 4], f32, tag="rlt")
            nc.sync.dma_start(rlt[:], rld[:, :])
            rl = [rlt[:, i : i + 1] for i in range(4)]
            half = cpool.tile([128, 1], f32, tag="half")
            nc.vector.memset(half[:], 0.5)

            for b in range(BPC):
                qt, xt = [], []
                for k in range(2):
                    t = ipool.tile([128, L], mmdt, tag=f"qt{k}")
                    nc.sync.dma_start(t[:], qT[b, 128 * k : 128 * (k + 1), :])
                    qt.append(t)
                    t = ipool.tile([128, L], mmdt, tag=f"xt{k}")
                    nc.sync.dma_start(t[:], xT[b, 128 * k : 128 * (k + 1), :])
                    xt.append(t)
                st = []
                for k in range(2):
                    t = ipool.tile([128, L], mmdt, tag=f"st{k}")
                    nc.sync.dma_start(t[:], sT[b, 128 * k : 128 * (k + 1), :])
                    st.append(t)

                dgall = ipool.tile([128, 4 * L], f32, tag="dgall")
                nc.sync.dma_start(
                    dgall[:].rearrange("p (t k) -> p t k", k=L),
                    dg[b].rearrange("(t p) k -> p t k", p=128),
                )
                czall = m1pool.tile([128, 4 * L], f32, tag="czall")
                tzall = m1pool.tile([128, 4 * L], f32, tag="tzall")
                nc.gpsimd.memset(czall[:], 0.0)
                nc.gpsimd.memset(tzall[:], 0.0)

                # projection order: cq ck tq tk qcc_q qcc_k qct_q qct_k
                # All projections run over the full (padded) 512 columns; the
                # s[:,1:] / s[:,:-1] shifts are applied when slicing the
                # projection outputs for the score matmuls.
                srcs = [qt, xt, qt, xt, st, st, st, st]
                projs = []
                for j in range(8):
                    pr = []
                    for mh in range(2):
                        ps = psB.tile([128, L], f32, tag="psproj")
                        nc.tensor.matmul(
                            ps[:, :],
                            mm(wtiles[j][0][:, 128 * mh : 128 * (mh + 1)]),
                            mm(srcs[j][0][:, :]),
                            start=True,
                            stop=False,
                        )
                        nc.tensor.matmul(
                            ps[:, :],
                            mm(wtiles[j][1][:, 128 * mh : 128 * (mh + 1)]),
                            mm(srcs[j][1][:, :]),
                            start=False,
                            stop=True,
                        )
                        sb = ppool.tile([128, L + 2], mmdt, tag=f"proj{j}{mh}")
                        if j in (1, 3, 5):
                            nc.vector.tensor_scalar_add(
                                sb[:, 0:L], ps[:], btiles[j][mh]
                            )
                        else:
                            nc.scalar.activation(
                                sb[:, 0:L], ps[:], AF.Identity,
                                bias=btiles[j][mh], scale=1.0,
                            )
                        nc.gpsimd.memset(sb[:, L : L + 2].bitcast(dt.uint32), 0)
                        pr.append(sb)
                    projs.append(pr)

                p_cq, p_ck, p_tq, p_tk, p_ccq, p_cck, p_ctq, p_ctk = projs

                for i, (rs, re, W) in enumerate(QTILES):
                    M = re - rs
                    Me = 128          # padded stationary width (rows computed)
                    We = (W + 1) & ~1  # even moving width

                    def score(ql, qoff, kl, tag):
                        ps = psA.tile([128, L], f32, tag=tag)
                        nc.tensor.matmul(
                            ps[:Me, :We],
                            mm(ql[0][:, rs + qoff : rs + qoff + Me]),
                            mm(kl[0][:, 0:We]),
                            start=True,
                            stop=False,
                        )
                        nc.tensor.matmul(
                            ps[:Me, :We],
                            mm(ql[1][:, rs + qoff : rs + qoff + Me]),
                            mm(kl[1][:, 0:We]),
                            start=False,
                            stop=True,
                        )
                        return ps

                    # Q/X-side projections index q directly; s-side q
                    # projections are shifted by one (q = s[1:] slice).
                    A_c = score(p_cq, 0, p_ck, "scA")
                    P_cc = score(p_ccq, 1, p_cck, "scB")
                    A_t = score(p_tq, 0, p_tk, "scC")
                    P_ct = score(p_ctq, 1, p_ctk, "scD")

                    pcc = m1pool.tile([128, L], f32, tag="pcc")
                    nc.scalar.activation(pcc[:M, :We], P_cc[:M, :We], AF.Copy)
                    pct = m1pool.tile([128, L], f32, tag="pct")
                    nc.vector.tensor_copy(pct[:M, :We], P_ct[:M, :We])

                    p0 = m1pool.tile([128, L], f32, tag="p0")
                    nc.vector.tensor_tensor(
                        p0[:M, :We], A_c[:M, :We], pcc[:M, :We], OP.mult
                    )
                    p1 = m1pool.tile([128, L], f32, tag="p1")
                    nc.vector.tensor_tensor(
                        p1[:M, :We], A_t[:M, :We], pct[:M, :We], OP.mult
                    )

                    # causal branch: mask -> -FLT_MAX, rowmax, exp+sum
                    p0m = m1pool.tile([128, L], f32, tag="p0m")
                    mx0 = spool.tile([128, 1], f32, tag="mx0")
                    nc.vector._custom_dve(
                        TMR,
                        out=p0m[:M, :We],
                        in0=p0[:M, :We],
                        in1=rl[i][:M, :],
                        s0=0.0,
                        s1=-3.4e38,
                        imm2=1.0,
                        accum_out=mx0[:M, :],
                    )
                    nmax0 = spool.tile([128, 1], f32, tag="nmax")
                    nc.vector.tensor_scalar_mul(nmax0[:M, :], mx0[:M, :], -1.0)
                    sums = spool.tile([128, 2], f32, tag="sums")
                    e0 = mpool.tile([128, L], f32, tag="e0")
                    nc.scalar.activation(
                        e0[:M, :We],
                        p0m[:M, :We],
                        AF.Exp,
                        bias=nmax0[:M, :],
                        scale=1.0,
                        accum_out=sums[:M, 0:1],
                    )

                    # trivial branch: 1-sigmoid(x) = 0.5 + 0.5*tanh(-x/2)
                    t1 = m1pool.tile([128, L], f32, tag="t1")
                    nc.scalar.activation(t1[:M, :We], p1[:M, :We], AF.Tanh, scale=-0.5)
                    p1m = m1pool.tile([128, L], f32, tag="p1m")
                    nc.vector._custom_dve(
                        TMR,
                        out=p1m[:M, :We],
                        in0=t1[:M, :We],
                        in1=rl[i][:M, :],
                        s0=0.0,
                        s1=-3.4e38,
                        imm2=0.5,
                    )
                    e1 = mpool.tile([128, L], f32, tag="e1")
                    nc.scalar.activation(
                        e1[:M, :We],
                        p1m[:M, :We],
                        AF.Exp,
                        bias=half[:M, :],
                        scale=1.0,
                        accum_out=sums[:M, 1:2],
                    )

                    rec = spool.tile([128, 2], f32, tag="rec")
                    nc.vector.reciprocal(rec[:M, :], sums[:M, :])

                    c1 = mpool.tile([128, L], f32, tag="c1")
                    nc.vector.tensor_scalar_mul(c1[:M, :We], e1[:M, :We], rec[:M, 1:2])

                    diff = m1pool.tile([128, L], f32, tag="diff")
                    nc.vector.scalar_tensor_tensor(
                        diff[:M, :We],
                        e0[:M, :We],
                        rec[:M, 0:1],
                        c1[:M, :We],
                        OP.mult,
                        OP.subtract,
                    )
                    dgt = dgall[:, i * L : i * L + We]
                    cz = czall[:, i * L : i * L + We]
                    tz = tzall[:, i * L : i * L + We]
                    nc.vector.tensor_tensor(
                        cz[:M, :], diff[:M, :We], dgt[:M, :], OP.is_ge
                    )
                    nc.vector.tensor_tensor(
                        tz[:M, :], m01[i][:M, :We], cz[:M, :], OP.subtract
                    )

                nc.sync.dma_start(
                    out_c[b].rearrange("(t p) k -> p t k", p=128),
                    czall[:].rearrange("p (t k) -> p t k", k=L),
                )
                nc.sync.dma_start(
                    out_t[b].rearrange("(t p) k -> p t k", p=128),
                    tzall[:].rearrange("p (t k) -> p t k", k=L),
                )

    nc.compile()
    return nc


def _get_nc(use_f32r: bool):
    key = ("nc", use_f32r)
    if key not in _CACHE:
        _CACHE[key] = _build_bass(use_f32r)
    return _CACHE[key]


def _gumbel_diff():
    """d[b,q,k] = g1 - g0 for the fixed key-42 gumbel draw of the reference."""
    if "gd" not in _CACHE:
        import jax

        g = jax.random.gumbel(jax.random.key(42), (B, S, 2, S), dtype=np.float32)
        _CACHE["gd"] = np.asarray(g[:, :, 1, :] - g[:, :, 0, :], dtype=np.float32)
    return _CACHE["gd"]


def _host_fallback(kw):
    """Exact reference recomputation on host (only used if the masks are not
    the expected causal/no-padding pattern)."""
    import jax
    import jax.numpy as jnp

    def qc_score(ques, conc, qw, qb, kw_, kb):
        s = ques + conc
        q = s[:, 1:] @ qw.T + qb
        k = s[:, :-1] @ kw_.T + kb
        return jnp.einsum("bqh,bkh->bqk", q, k) * SCALE

    def masked(a, att, kpm):
        neg = jnp.float32(-jnp.inf)
        a = jnp.where(att, neg, a)
        return jnp.where(kpm[:, None, :], neg, a)

    att, kpm = kw["att_mask"], kw["key_padding_mask"]
    qc_c = qc_score(kw["ques_state"], kw["conc_state"], kw["qcc_q_w"], kw["qcc_q_b"],
                    kw["qcc_k_w"], kw["qcc_k_b"])
    qc_t = qc_score(kw["ques_state"], kw["conc_state"], kw["qct_q_w"], kw["qct_q_b"],
                    kw["qct_k_w"], kw["qct_k_b"])
    qc = kw["Q_state"] @ kw["cq_w"].T + kw["cq_b"]
    kc = kw["X_state"] @ kw["ck_w"].T + kw["ck_b"]
    ac = jnp.einsum("bqh,bkh->bqk", qc, kc) * SCALE * qc_c
    c_score = jax.nn.softmax(masked(ac, att, kpm), axis=-1)
    qt = kw["Q_state"] @ kw["tq_w"].T + kw["tq_b"]
    kt = kw["X_state"] @ kw["tk_w"].T + kw["tk_b"]
    at = jnp.einsum("bqh,bkh->bqk", qt, kt) * SCALE * qc_t
    at = 1.0 - jax.nn.sigmoid(at)
    t_score = jax.nn.softmax(masked(at, att, kpm), axis=-1)
    score = jnp.stack([c_score, t_score], axis=2)
    g = jax.random.gumbel(jax.random.key(42), score.shape, dtype=score.dtype)
    y = jax.nn.softmax(score + g, axis=2)
    idx = jnp.argmax(y, axis=2)
    causal = jnp.where(att, 0.0, (idx == 0).astype(np.float32))
    trivial = jnp.where(att, 0.0, (idx == 1).astype(np.float32))
    return np.asarray(causal), np.asarray(trivial)


def kernel(**inputs):
    import os

    Q = np.asarray(inputs["Q_state"], dtype=np.float32)
    X = np.asarray(inputs["X_state"], dtype=np.float32)
    ques = np.asarray(inputs["ques_state"], dtype=np.float32)
    conc = np.asarray(inputs["conc_state"], dtype=np.float32)
    att = np.asarray(inputs["att_mask"])
    kpm = np.asarray(inputs["key_padding_mask"])

    triu = np.triu(np.ones((S, S), dtype=bool), k=1)
    if kpm.any() or not (att == triu[None]).all():
        return _host_fallback(inputs)

    use_f32r = os.environ.get("KBENCH_F32R", "0") == "1"
    trace = os.environ.get("KBENCH_TRACE", "0") == "1"
    _install_ntff_hook()

    names = ["cq", "ck", "tq", "tk", "qcc_q", "qcc_k", "qct_q", "qct_k"]
    q_side = {"cq", "tq", "qcc_q", "qct_q"}
    wts = np.empty((8, H, H), np.float32)
    bss = np.empty((128, 16), np.float32)  # [p, 2j+mh] = b_j[128*mh + p]
    for j, nm in enumerate(names):
        w = np.asarray(inputs[nm + "_w"], dtype=np.float32)
        bvec = np.asarray(inputs[nm + "_b"], dtype=np.float32)
        if nm in q_side:
            w = w * SCALE
            bvec = bvec * SCALE
        wts[j] = np.ascontiguousarray(w.T)  # [h_in, h_out]
        bss[:, 2 * j] = bvec[0:128]
        bss[:, 2 * j + 1] = bvec[128:256]

    # masks / rowlen consts (padded to L columns)
    m01 = np.zeros((4, 128, L), np.float32)
    rld = np.zeros((128, 4), np.float32)
    for i, (rs, re, W) in enumerate(QTILES):
        for p in range(re - rs):
            q = rs + p
            m01[i, p, : q + 1] = 1.0
            rld[p, i] = q + 1

    d = _gumbel_diff()
    s_full = ques + conc

    qT = np.zeros((B, H, L), np.float32)
    qT[:, :, :S] = Q.transpose(0, 2, 1)
    xT = np.zeros((B, H, L), np.float32)
    xT[:, :, :S] = X.transpose(0, 2, 1)
    sT = np.ascontiguousarray(s_full.transpose(0, 2, 1))
    dgm = np.full((B, L, L), BIG, np.float32)
    dgm[:, :S, :S] = np.where(triu[None], BIG, d)

    nc = _get_nc(use_f32r)
    from concourse.bass_utils import run_bass_kernel_spmd

    in_maps = []
    for c in range(NCORE):
        sl = slice(c * BPC, (c + 1) * BPC)
        in_maps.append(
            {
                "qT": qT[sl],
                "xT": xT[sl],
                "sT": sT[sl],
                "dg": dgm[sl],
                "wts": wts,
                "bss": bss,
                "m01d": m01,
                "rld": rld,
            }
        )

    kwargs = {}
    if trace:
        n = _CACHE.get("trace_n", 0)
        _CACHE["trace_n"] = n + 1
        tdir = os.environ.get("KBENCH_TRACE_DIR", "/tmp/kbench_trace") + f"_{n}"
        os.makedirs(tdir, exist_ok=True)
        kwargs = {"trace": True, "tmpdir": tdir}
        _CACHE["trace_dir"] = tdir
    res = run_bass_kernel_spmd(nc, in_maps, core_ids=list(range(NCORE)), **kwargs)
    if trace:
        _CACHE["exec_time_ns"] = res.exec_time_ns
        _CACHE["mean_exec_time_ns"] = res.mean_exec_time_ns

    causal = np.empty((B, S, S), np.float32)
    trivial = np.empty((B, S, S), np.float32)
    for c in range(NCORE):
        sl = slice(c * BPC, (c + 1) * BPC)
        causal[sl] = res.results[c]["out_c"][:, :S, :S]
        trivial[sl] = res.results[c]["out_t"][:, :S, :S]
    return causal, trivial


# revision 24
# speedup vs baseline: 2.1049x; 1.1862x over previous
"""Trainium2 Bass kernel for nn_Disentangle_causal.

Math (per batch b):
  s  = ques + conc                               [L, H]
  qP = s[1:] @ Wq.T + bq  (for qcc / qct pairs)  [S, H]
  qc = Q @ cq_w.T + cq_b, kc = X @ ck_w.T + ck_b, etc.
  A_c  = (qc kc^T) * SCALE ; P_cc = (qcc_q qcc_k^T) * SCALE
  ac   = A_c * P_cc        -> causal_score  = softmax(mask(ac))
  A_t, P_ct likewise; at = 1 - sigmoid(A_t * P_ct) -> trivial_score
  g ~ gumbel(key 42, [B,S,2,S]);  idx = argmax_i(score_i + g_i)
  causal_mask = (idx==0) & ~mask ; trivial_mask = (idx==1) & ~mask

Device reformulation:
  1 - sigmoid(x) = 0.5 - 0.5*tanh(x/2)  (tanh shares ACT table set with exp)
  causal = ((e0/s0 - e1/s1) >= d) with d = g1-g0 (masked entries -> +BIG)
  trivial = M01 - causal  (M01 = 0/1 lower-tri mask)
SCALE is folded into the q-side weights/biases on the host. Only the
lower-triangular block-columns are computed; outputs are pre-zeroed by
the runtime, so masked blocks are never touched.
"""

import sys
import types

import numpy as np

if "/opt/trn_rl_repo" not in sys.path:
    sys.path.insert(0, "/opt/trn_rl_repo")

B, L, H = 64, 512, 256
S = L - 1
NCORE = 8
BPC = B // NCORE  # batches per core
SCALE = float(H) ** -0.5
BIG = np.float32(1.0e30)

# q-row tiles: (row_start, row_end, active_width)
QTILES = [(0, 128, 128), (128, 256, 256), (256, 384, 384), (384, 511, 511)]

_CACHE: dict = {}


def _install_ntff_hook():
    """Make trace=True work under axon (antenv.axon_hooks is not shipped)."""
    try:
        import antenv

        if "antenv.axon_hooks" in sys.modules:
            return
        hooks = types.ModuleType("antenv.axon_hooks")
        _hook = [None]
        hooks.set_axon_ntff_profile_hook = lambda h: _hook.__setitem__(0, h)
        hooks.get_axon_ntff_profile_hook = lambda: _hook[0]
        sys.modules["antenv.axon_hooks"] = hooks
        antenv.axon_hooks = hooks
        from trn_agent_boot.trn_boot import _ntff_profile_via_ctypes

        hooks.set_axon_ntff_profile_hook(
            _ntff_profile_via_ctypes("/opt/axon/libaxon_pjrt.so")
        )
    except Exception:
        pass


def _build_bass(use_f32r: bool):
    import concourse.mybir as mybir
    import concourse.tile as tile
    from concourse.bacc import Bacc
    from concourse.dve_ops import TENSOR_MASK_REDUCE as TMR

    dt = mybir.dt
    f32 = dt.float32
    AF = mybir.ActivationFunctionType
    OP = mybir.AluOpType

    nc = Bacc("TRN2", target_bir_lowering=False)

    mmdt = dt.float32r if use_f32r else f32
    qT = nc.dram_tensor("qT", [BPC, H, L], mmdt, kind="ExternalInput")
    xT = nc.dram_tensor("xT", [BPC, H, L], mmdt, kind="ExternalInput")
    sT = nc.dram_tensor("sT", [BPC, H, L], mmdt, kind="ExternalInput")
    dg = nc.dram_tensor("dg", [BPC, L, L], f32, kind="ExternalInput")
    wts = nc.dram_tensor("wts", [8, H, H], mmdt, kind="ExternalInput")
    bss = nc.dram_tensor("bss", [128, 16], f32, kind="ExternalInput")
    m01d = nc.dram_tensor("m01d", [4, 128, L], dt.bfloat16, kind="ExternalInput")
    rld = nc.dram_tensor("rld", [128, 4], f32, kind="ExternalInput")
    out_c = nc.dram_tensor("out_c", [BPC, L, L], dt.bfloat16, kind="ExternalOutput")
    out_t = nc.dram_tensor("out_t", [BPC, L, L], dt.bfloat16, kind="ExternalOutput")

    def mm(ap):
        return ap

    with tile.TileContext(nc) as tc:
        with (
            tc.tile_pool(name="consts", bufs=1) as cpool,
            tc.tile_pool(name="inp", bufs=2) as ipool,
            tc.tile_pool(name="proj", bufs=2) as ppool,
            tc.tile_pool(name="mid", bufs=2) as mpool,
            tc.tile_pool(name="mid1", bufs=1) as m1pool,
            tc.tile_pool(name="small", bufs=3) as spool,
            tc.tile_pool(name="psA", bufs=1, space="PSUM") as psA,
            tc.tile_pool(name="psB", bufs=3, space="PSUM") as psB,
        ):
            wtiles = []
            for j in range(8):
                w0 = cpool.tile([128, H], mmdt, tag=f"w{j}0")
                nc.sync.dma_start(w0[:], wts[j, 0:128, :])
                w1 = cpool.tile([128, H], mmdt, tag=f"w{j}1")
                nc.sync.dma_start(w1[:], wts[j, 128:256, :])
                wtiles.append((w0, w1))
            bpk = cpool.tile([128, 16], f32, tag="bpk")
            nc.sync.dma_start(bpk[:], bss[:, :])
            btiles = [(bpk[:, 2 * j : 2 * j + 1], bpk[:, 2 * j + 1 : 2 * j + 2])
                      for j in range(8)]
            m01 = []
            for i in range(4):
                m = cpool.tile([128, L], dt.bfloat16, tag=f"m01_{i}")
                nc.sync.dma_start(m[:], m01d[i])
                m01.append(m)
            rlt = cpool.tile([128, 4], f32, tag="rlt")
            nc.sync.dma_start(rlt[:], rld[:, :])
            rl = [rlt[:, i : i + 1] for i in range(4)]
            half = cpool.tile([128, 1], f32, tag="half")
            nc.vector.memset(half[:], 0.5)

            for b in range(BPC):
                qt, xt = [], []
                for k in range(2):
                    t = ipool.tile([128, L], mmdt, tag=f"qt{k}")
                    nc.sync.dma_start(t[:], qT[b, 128 * k : 128 * (k + 1), :])
                    qt.append(t)
                    t = ipool.tile([128, L], mmdt, tag=f"xt{k}")
                    nc.sync.dma_start(t[:], xT[b, 128 * k : 128 * (k + 1), :])
                    xt.append(t)
                st = []
                for k in range(2):
                    t = ipool.tile([128, L], mmdt, tag=f"st{k}")
                    nc.sync.dma_start(t[:], sT[b, 128 * k : 128 * (k + 1), :])
                    st.append(t)

                dgall = ipool.tile([128, 4 * L], f32, tag="dgall")
                nc.sync.dma_start(
                    dgall[:].rearrange("p (t k) -> p t k", k=L),
                    dg[b].rearrange("(t p) k -> p t k", p=128),
                )
                czall = m1pool.tile([128, 4 * L], dt.bfloat16, tag="czall")
                tzall = m1pool.tile([128, 4 * L], dt.bfloat16, tag="tzall")
                nc.gpsimd.memset(czall[:], 0.0)
                nc.gpsimd.memset(tzall[:], 0.0)

                # projection order: cq ck tq tk qcc_q qcc_k qct_q qct_k
                # All projections run over the full (padded) 512 columns; the
                # s[:,1:] / s[:,:-1] shifts are applied when slicing the
                # projection outputs for the score matmuls.
                srcs = [qt, xt, qt, xt, st, st, st, st]
                projs = []
                for j in range(8):
                    pr = []
                    for mh in range(2):
                        ps = psB.tile([128, L], f32, tag="psproj")
                        nc.tensor.matmul(
                            ps[:, :],
                            mm(wtiles[j][0][:, 128 * mh : 128 * (mh + 1)]),
                            mm(srcs[j][0][:, :]),
                            start=True,
                            stop=False,
                        )
                        nc.tensor.matmul(
                            ps[:, :],
                            mm(wtiles[j][1][:, 128 * mh : 128 * (mh + 1)]),
                            mm(srcs[j][1][:, :]),
                            start=False,
                            stop=True,
                        )
                        sb = ppool.tile([128, L + 2], mmdt, tag=f"proj{j}{mh}")
                        if (2 * j + mh) % 2 == 1:
                            nc.vector.tensor_scalar_add(
                                sb[:, 0:L], ps[:], btiles[j][mh]
                            )
                        else:
                            nc.scalar.activation(
                                sb[:, 0:L], ps[:], AF.Identity,
                                bias=btiles[j][mh], scale=1.0,
                            )
                        nc.gpsimd.memset(sb[:, L : L + 2].bitcast(dt.uint32), 0)
                        pr.append(sb)
                    projs.append(pr)

                p_cq, p_ck, p_tq, p_tk, p_ccq, p_cck, p_ctq, p_ctk = projs

                for i, (rs, re, W) in enumerate(QTILES):
                    M = re - rs
                    Me = 128          # padded stationary width (rows computed)
                    We = (W + 1) & ~1  # even moving width

                    def score(ql, qoff, kl, tag):
                        ps = psA.tile([128, L], f32, tag=tag)
                        nc.tensor.matmul(
                            ps[:Me, :We],
                            mm(ql[0][:, rs + qoff : rs + qoff + Me]),
                            mm(kl[0][:, 0:We]),
                            start=True,
                            stop=False,
                        )
                        nc.tensor.matmul(
                            ps[:Me, :We],
                            mm(ql[1][:, rs + qoff : rs + qoff + Me]),
                            mm(kl[1][:, 0:We]),
                            start=False,
                            stop=True,
                        )
                        return ps

                    # Q/X-side projections index q directly; s-side q
                    # projections are shifted by one (q = s[1:] slice).
                    A_c = score(p_cq, 0, p_ck, "scA")
                    P_cc = score(p_ccq, 1, p_cck, "scB")
                    A_t = score(p_tq, 0, p_tk, "scC")
                    P_ct = score(p_ctq, 1, p_ctk, "scD")

                    pcc = m1pool.tile([128, L], f32, tag="pcc")
                    nc.scalar.activation(pcc[:M, :We], P_cc[:M, :We], AF.Copy)
                    pct = m1pool.tile([128, L], f32, tag="pct")
                    nc.scalar.activation(pct[:M, :We], P_ct[:M, :We], AF.Copy)

                    p0 = m1pool.tile([128, L], f32, tag="p0")
                    nc.vector.tensor_tensor(
                        p0[:M, :We], A_c[:M, :We], pcc[:M, :We], OP.mult
                    )
                    p1 = m1pool.tile([128, L], f32, tag="p1")
                    nc.vector.tensor_tensor(
                        p1[:M, :We], A_t[:M, :We], pct[:M, :We], OP.mult
                    )

                    # causal branch: mask -> -FLT_MAX, rowmax, exp+sum
                    p0m = m1pool.tile([128, L], f32, tag="p0m")
                    mx0 = spool.tile([128, 1], f32, tag="mx0")
                    nc.vector._custom_dve(
                        TMR,
                        out=p0m[:M, :We],
                        in0=p0[:M, :We],
                        in1=rl[i][:M, :],
                        s0=0.0,
                        s1=-3.4e38,
                        imm2=1.0,
                        accum_out=mx0[:M, :],
                    )
                    nmax0 = spool.tile([128, 1], f32, tag="nmax")
                    nc.vector.tensor_scalar_mul(nmax0[:M, :], mx0[:M, :], -1.0)
                    sums = spool.tile([128, 2], f32, tag="sums")
                    e0 = mpool.tile([128, L], f32, tag="e0")
                    nc.scalar.activation(
                        e0[:M, :We],
                        p0m[:M, :We],
                        AF.Exp,
                        bias=nmax0[:M, :],
                        scale=1.0,
                        accum_out=sums[:M, 0:1],
                    )

                    # trivial branch: 1-sigmoid(x) = 0.5 + 0.5*tanh(-x/2)
                    t1 = m1pool.tile([128, L], f32, tag="t1")
                    nc.scalar.activation(t1[:M, :We], p1[:M, :We], AF.Tanh, scale=-0.5)
                    p1m = m1pool.tile([128, L], f32, tag="p1m")
                    nc.vector._custom_dve(
                        TMR,
                        out=p1m[:M, :We],
                        in0=t1[:M, :We],
                        in1=rl[i][:M, :],
                        s0=0.0,
                        s1=-3.4e38,
                        imm2=0.5,
                    )
                    e1 = mpool.tile([128, L], f32, tag="e1")
                    nc.scalar.activation(
                        e1[:M, :We],
                        p1m[:M, :We],
                        AF.Exp,
                        bias=half[:M, :],
                        scale=1.0,
                        accum_out=sums[:M, 1:2],
                    )

                    rec = spool.tile([128, 2], f32, tag="rec")
                    nc.vector.reciprocal(rec[:M, :], sums[:M, :])

                    dgt = dgall[:, i * L : i * L + We]
                    cz = czall[:, i * L : i * L + We]
                    tz = tzall[:, i * L : i * L + We]
                    zz = m1pool.tile([128, L], f32, tag="zz")
                    nc.vector.scalar_tensor_tensor(
                        zz[:M, :We],
                        e1[:M, :We],
                        rec[:M, 1:2],
                        dgt[:M, :],
                        OP.mult,
                        OP.add,
                    )
                    nc.vector.scalar_tensor_tensor(
                        cz[:M, :],
                        e0[:M, :We],
                        rec[:M, 0:1],
                        zz[:M, :We],
                        OP.mult,
                        OP.is_ge,
                    )
                    nc.vector.tensor_tensor(
                        tz[:M, :], m01[i][:M, :We], cz[:M, :], OP.subtract
                    )

                nc.sync.dma_start(
                    out_c[b].rearrange("(t p) k -> p t k", p=128),
                    czall[:].rearrange("p (t k) -> p t k", k=L),
                )
                nc.sync.dma_start(
                    out_t[b].rearrange("(t p) k -> p t k", p=128),
                    tzall[:].rearrange("p (t k) -> p t k", k=L),
                )

    nc.compile()
    return nc


def _get_nc(use_f32r: bool):
    key = ("nc", use_f32r)
    if key not in _CACHE:
        _CACHE[key] = _build_bass(use_f32r)
    return _CACHE[key]


def _gumbel_diff():
    """d[b,q,k] = g1 - g0 for the fixed key-42 gumbel draw of the reference."""
    if "gd" not in _CACHE:
        import jax

        g = jax.random.gumbel(jax.random.key(42), (B, S, 2, S), dtype=np.float32)
        _CACHE["gd"] = np.asarray(g[:, :, 1, :] - g[:, :, 0, :], dtype=np.float32)
    return _CACHE["gd"]


def _host_fallback(kw):
    """Exact reference recomputation on host (only used if the masks are not
    the expected causal/no-padding pattern)."""
    import jax
    import jax.numpy as jnp

    def qc_score(ques, conc, qw, qb, kw_, kb):
        s = ques + conc
        q = s[:, 1:] @ qw.T + qb
        k = s[:, :-1] @ kw_.T + kb
        return jnp.einsum("bqh,bkh->bqk", q, k) * SCALE

    def masked(a, att, kpm):
        neg = jnp.float32(-jnp.inf)
        a = jnp.where(att, neg, a)
        return jnp.where(kpm[:, None, :], neg, a)

    att, kpm = kw["att_mask"], kw["key_padding_mask"]
    qc_c = qc_score(kw["ques_state"], kw["conc_state"], kw["qcc_q_w"], kw["qcc_q_b"],
                    kw["qcc_k_w"], kw["qcc_k_b"])
    qc_t = qc_score(kw["ques_state"], kw["conc_state"], kw["qct_q_w"], kw["qct_q_b"],
                    kw["qct_k_w"], kw["qct_k_b"])
    qc = kw["Q_state"] @ kw["cq_w"].T + kw["cq_b"]
    kc = kw["X_state"] @ kw["ck_w"].T + kw["ck_b"]
    ac = jnp.einsum("bqh,bkh->bqk", qc, kc) * SCALE * qc_c
    c_score = jax.nn.softmax(masked(ac, att, kpm), axis=-1)
    qt = kw["Q_state"] @ kw["tq_w"].T + kw["tq_b"]
    kt = kw["X_state"] @ kw["tk_w"].T + kw["tk_b"]
    at = jnp.einsum("bqh,bkh->bqk", qt, kt) * SCALE * qc_t
    at = 1.0 - jax.nn.sigmoid(at)
    t_score = jax.nn.softmax(masked(at, att, kpm), axis=-1)
    score = jnp.stack([c_score, t_score], axis=2)
    g = jax.random.gumbel(jax.random.key(42), score.shape, dtype=score.dtype)
    y = jax.nn.softmax(score + g, axis=2)
    idx = jnp.argmax(y, axis=2)
    causal = jnp.where(att, 0.0, (idx == 0).astype(np.float32))
    trivial = jnp.where(att, 0.0, (idx == 1).astype(np.float32))
    return np.asarray(causal), np.asarray(trivial)


def kernel(**inputs):
    import os

    Q = np.asarray(inputs["Q_state"], dtype=np.float32)
    X = np.asarray(inputs["X_state"], dtype=np.float32)
    ques = np.asarray(inputs["ques_state"], dtype=np.float32)
    conc = np.asarray(inputs["conc_state"], dtype=np.float32)
    att = np.asarray(inputs["att_mask"])
    kpm = np.asarray(inputs["key_padding_mask"])

    triu = np.triu(np.ones((S, S), dtype=bool), k=1)
    if kpm.any() or not (att == triu[None]).all():
        return _host_fallback(inputs)

    use_f32r = os.environ.get("KBENCH_F32R", "0") == "1"
    trace = os.environ.get("KBENCH_TRACE", "0") == "1"
    _install_ntff_hook()

    names = ["cq", "ck", "tq", "tk", "qcc_q", "qcc_k", "qct_q", "qct_k"]
    q_side = {"cq", "tq", "qcc_q", "qct_q"}
    wts = np.empty((8, H, H), np.float32)
    bss = np.empty((128, 16), np.float32)  # [p, 2j+mh] = b_j[128*mh + p]
    for j, nm in enumerate(names):
        w = np.asarray(inputs[nm + "_w"], dtype=np.float32)
        bvec = np.asarray(inputs[nm + "_b"], dtype=np.float32)
        if nm in q_side:
            w = w * SCALE
            bvec = bvec * SCALE
        wts[j] = np.ascontiguousarray(w.T)  # [h_in, h_out]
        bss[:, 2 * j] = bvec[0:128]
        bss[:, 2 * j + 1] = bvec[128:256]

    import ml_dtypes

    # masks / rowlen consts (padded to L columns)
    m01 = np.zeros((4, 128, L), ml_dtypes.bfloat16)
    rld = np.zeros((128, 4), np.float32)
    for i, (rs, re, W) in enumerate(QTILES):
        for p in range(re - rs):
            q = rs + p
            m01[i, p, : q + 1] = 1.0
            rld[p, i] = q + 1

    d = _gumbel_diff()
    s_full = ques + conc

    qT = np.zeros((B, H, L), np.float32)
    qT[:, :, :S] = Q.transpose(0, 2, 1)
    xT = np.zeros((B, H, L), np.float32)
    xT[:, :, :S] = X.transpose(0, 2, 1)
    sT = np.ascontiguousarray(s_full.transpose(0, 2, 1))
    dgm = np.full((B, L, L), BIG, np.float32)
    dgm[:, :S, :S] = np.where(triu[None], BIG, d)

    nc = _get_nc(use_f32r)
    from concourse.bass_utils import run_bass_kernel_spmd

    in_maps = []
    for c in range(NCORE):
        sl = slice(c * BPC, (c + 1) * BPC)
        in_maps.append(
            {
                "qT": qT[sl],
                "xT": xT[sl],
                "sT": sT[sl],
                "dg": dgm[sl],
                "wts": wts,
                "bss": bss,
                "m01d": m01,
                "rld": rld,
            }
        )

    kwargs = {}
    if trace:
        n = _CACHE.get("trace_n", 0)
        _CACHE["trace_n"] = n + 1
        tdir = os.environ.get("KBENCH_TRACE_DIR", "/tmp/kbench_trace") + f"_{n}"
        os.makedirs(tdir, exist_ok=True)
        kwargs = {"trace": True, "tmpdir": tdir}
        _CACHE["trace_dir"] = tdir
    res = run_bass_kernel_spmd(nc, in_maps, core_ids=list(range(NCORE)), **kwargs)
    if trace:
        _CACHE["exec_time_ns"] = res.exec_time_ns
        _CACHE["mean_exec_time_ns"] = res.mean_exec_time_ns

    causal = np.empty((B, S, S), np.float32)
    trivial = np.empty((B, S, S), np.float32)
    for c in range(NCORE):
        sl = slice(c * BPC, (c + 1) * BPC)
        causal[sl] = res.results[c]["out_c"][:, :S, :S].astype(np.float32)
        trivial[sl] = res.results[c]["out_t"][:, :S, :S].astype(np.float32)
    return causal, trivial
